# revision 23
# baseline (speedup 1.0000x reference)
"""GRU-D forward on 8 Trainium2 NeuronCores (Bass/Tile kernel).

Key algebraic structure exploited:
  - The gates z_t, h~_t depend only on inputs (not on h), so all matmuls are
    parallel over T; only the elementwise blend h = (1-z)h + z*h~ is a
    recurrence, and it maps onto the DVE tensor_tensor_scan instruction
    (state = a*state - c' with a = sigmoid(-pre_z), c' = (a-1)*h~).
  - r_t is computed-but-unused in the reference -> Wr matmul skipped.
  - The xm block of inp = [x_tilde, xm, m] is constant -> folded into biases.

Sharding: data-parallel over batch (64 rows per core); weights replicated.
"""

import os
import sys

import numpy as np

sys.path.insert(0, "/opt/trn_rl_repo")

B, T, D, H = 512, 256, 256, 1024
NC = 8
BL = B // NC  # 64 batch rows per core
SB = 2  # batch elems per sub-batch (x T=256 -> 512 matmul rows)
NSB = BL // SB  # 32 sub-batches
KT = 4  # K tiles of 128 over 2D=512 contraction
K2 = 2  # fp8 DoubleRow K tiles of 256
I2 = 2  # DoubleRow plane pairing
HT = 8  # H tiles of 128
P = 128

# Engine/dtype configuration of the graded kernel.
MAIN_PARTS = ("mm", "act", "dve", "fp8")

_nc_cache = {}
_last_results = None


def _build_bass(repeat=1, parts=("mm", "act", "dve")):
    """Build the Bass program. repeat>1 wraps the whole computation in a
    hardware For loop executing it `repeat` times — used only for timing
    (overhead-cancelling slope measurement); the graded path uses repeat=1.
    parts: subset of {mm, act, dve} — engine-attribution experiments."""
    parts = frozenset(parts)
    key = (repeat, parts)
    if key in _nc_cache:
        return _nc_cache[key]
    from contextlib import ExitStack

    from concourse import bacc, mybir, tile

    nc = bacc.Bacc("TRN2", target_bir_lowering=False, debug=False, num_devices=NC)
    bf16 = mybir.dt.bfloat16
    f32 = mybir.dt.float32
    AF = mybir.ActivationFunctionType
    OP = mybir.AluOpType

    f8 = mybir.dt.float8e4
    if "fp8" in parts:
        in_d = nc.dram_tensor(
            "inp8", [P, NSB, K2, I2, T, SB], f8, kind="ExternalInput"
        )
        w_d = nc.dram_tensor("w8", [P, 2, K2, HT, I2, P], f8, kind="ExternalInput")
    else:
        in_d = nc.dram_tensor("inp", [P, NSB, KT, T, SB], bf16, kind="ExternalInput")
        w_d = nc.dram_tensor("w", [P, 2, KT, HT, P], bf16, kind="ExternalInput")
    nbz_d = nc.dram_tensor("nbz", [P, HT], f32, kind="ExternalInput")
    bh_d = nc.dram_tensor("bh", [P, HT], f32, kind="ExternalInput")
    hout_d = nc.dram_tensor("hout", [P, HT, BL], f32, kind="ExternalOutput")

    with tile.TileContext(nc) as tc:
        with (
            tc.tile_pool(name="const", bufs=1) as cpool,
            tc.tile_pool(name="inb", bufs=3) as ipool,
            tc.tile_pool(name="act", bufs=2) as apool,
            tc.tile_pool(name="zps", bufs=2, space="PSUM") as zpool,
            tc.tile_pool(name="hps", bufs=2, space="PSUM") as hpool,
        ):
            if "fp8" in parts:
                w_s = cpool.tile([P, 2, K2, HT, I2, P], f8)
            else:
                w_s = cpool.tile([P, 2, KT, HT, P], bf16)
            nbz_s = cpool.tile([P, HT], f32)
            bh_s = cpool.tile([P, HT], f32)
            hlast = cpool.tile([P, HT, BL], f32)
            if "dve" not in parts:
                nc.vector.memset(hlast[:], 0.0)
            nc.sync.dma_start(w_s[:], w_d[:])
            nc.sync.dma_start(nbz_s[:], nbz_d[:])
            nc.sync.dma_start(bh_s[:], bh_d[:])

            with ExitStack() as rep_ctx:
                if repeat > 1:
                    rep_ctx.enter_context(tc.For_i(0, repeat, 1))
                _emit_body(nc, mybir, parts, ipool, apool, zpool, hpool,
                           in_d, w_s, nbz_s, bh_s, hlast)
            nc.sync.dma_start(hout_d[:], hlast[:])
    nc.compile()
    _nc_cache[key] = nc
    return nc


def _emit_body(nc, mybir, parts, ipool, apool, zpool, hpool, in_d, w_s, nbz_s, bh_s, hlast):
    bf16 = mybir.dt.bfloat16
    f8 = mybir.dt.float8e4
    f32 = mybir.dt.float32
    AF = mybir.ActivationFunctionType
    OP = mybir.AluOpType
    fp8 = "fp8" in parts
    for sb in range(NSB):
        if fp8:
            in_s = ipool.tile([P, K2, I2, T * SB], f8, tag="in")
        else:
            in_s = ipool.tile([P, KT, T, SB], bf16, tag="in")
        nc.sync.dma_start(in_s[:], in_d[:, sb])
        a_s = apool.tile([P, HT, T, SB], bf16, tag="a")
        ht_s = apool.tile([P, HT, T, SB], bf16, tag="ht")
        cp_s = apool.tile([P, HT, T, SB], bf16, tag="cp")
        so_s = apool.tile([P, HT, T, SB], f32, tag="so")
        if "flat" in parts:  # timing-only: 1-D free APs for matmul stream
            in_f = ipool.tile([P, KT, T * SB], bf16, tag="inf")
            nc.sync.dma_start(in_f[:], in_d[:, sb])
            for j in range(HT):
                ps_zf = zpool.tile([P, T * SB], f32, tag="zf")
                ps_hf = hpool.tile([P, T * SB], f32, tag="hf")
                for k in range(KT):
                    nc.tensor.matmul(
                        ps_zf[:], w_s[:, 0, k, j, :], in_f[:, k],
                        start=(k == 0), stop=(k == KT - 1),
                    )
                for k in range(KT):
                    nc.tensor.matmul(
                        ps_hf[:], w_s[:, 1, k, j, :], in_f[:, k],
                        start=(k == 0), stop=(k == KT - 1),
                    )
            continue
        for j in range(HT):
            ps_z = zpool.tile([P, T, SB], f32, tag="z")
            ps_h = hpool.tile([P, T, SB], f32, tag="h")
            if fp8 and "mm" in parts:
                DR = mybir.MatmulPerfMode.DoubleRow
                for gate, ps in ((0, ps_z), (1, ps_h)):
                    for k2 in range(K2):
                        nc.tensor.matmul(
                            ps[:],
                            w_s[:, gate, k2, j],
                            in_s[:, k2],
                            start=(k2 == 0),
                            stop=(k2 == K2 - 1),
                            perf_mode=DR,
                        )
            elif "mm" in parts:
                fixed_w = "w0" in parts  # timing-only: no weight switching
                bigacc = "bigacc" in parts  # timing-only: one huge accum group
                for k in range(KT):
                    nc.tensor.matmul(
                        ps_z[:],
                        w_s[:, 0, 0, 0, :] if fixed_w else w_s[:, 0, k, j, :],
                        in_s[:, k],
                        start=(j == 0 and k == 0) if bigacc else (k == 0),
                        stop=(j == HT - 1 and k == KT - 1) if bigacc else (k == KT - 1),
                        skip_group_check=bigacc,
                    )
                for k in range(KT):
                    nc.tensor.matmul(
                        ps_h[:],
                        w_s[:, 0, 0, 0, :] if fixed_w else w_s[:, 1, k, j, :],
                        in_s[:, k],
                        start=(j == 0 and k == 0) if bigacc else (k == 0),
                        stop=(j == HT - 1 and k == KT - 1) if bigacc else (k == KT - 1),
                        skip_group_check=bigacc,
                    )
            if "act" in parts:
                # a = 1 - z = sigmoid(-(pre_z + bz))
                nc.scalar.activation(
                    a_s[:, j], ps_z[:], AF.Sigmoid, bias=nbz_s[:, j : j + 1], scale=-1.0
                )
                nc.scalar.activation(
                    ht_s[:, j], ps_h[:], AF.Tanh, bias=bh_s[:, j : j + 1], scale=1.0
                )
            if "dve" in parts:
                # c' = (a - 1) * h~   (so that a*h - c' = a*h + (1-a)*h~)
                nc.vector.scalar_tensor_tensor(
                    cp_s[:, j], a_s[:, j], 1.0, ht_s[:, j], op0=OP.subtract, op1=OP.mult
                )
                for b in range(SB):
                    nc.vector.tensor_tensor_scan(
                        so_s[:, j, :, b],
                        a_s[:, j, :, b],
                        cp_s[:, j, :, b],
                        0.0,
                        op0=OP.mult,
                        op1=OP.subtract,
                    )
                nc.vector.tensor_copy(
                    hlast[:, j, sb * SB : (sb + 1) * SB], so_s[:, j, T - 1, :]
                )


def _prepare_in_maps(X, M, input_means, gamma_x, Wz, bz, Wh, bh, modes=("fp8",)):
    import ml_dtypes

    from concourse import mybir

    bf16 = ml_dtypes.bfloat16
    f8np = mybir.dt.np(mybir.dt.float8e4)
    X = np.asarray(X, np.float32)
    M = np.asarray(M, np.float32)
    xm = np.asarray(input_means, np.float32)
    gx = np.asarray(gamma_x, np.float32)
    Wz = np.asarray(Wz, np.float32)
    Wh = np.asarray(Wh, np.float32)
    bz = np.asarray(bz, np.float32)
    bhv = np.asarray(bh, np.float32)

    # x_tilde (exact, fp32, handles arbitrary gamma_x / non-binary M)
    g = np.exp(-gx * (1.0 - M))
    x_hat = M * X + (1.0 - M) * xm
    x_tilde = g * x_hat + (1.0 - g) * xm  # [B, T, D]

    # fold the constant xm block into the biases; drop unused Wr entirely
    Wz_eff = np.concatenate([Wz[:, :D], Wz[:, 2 * D :]], axis=1).T  # [2D, H]
    Wh_eff = np.concatenate([Wh[:, :D], Wh[:, 2 * D :]], axis=1).T
    bz_eff = bz + xm @ Wz[:, D : 2 * D].T
    bh_eff = bhv + xm @ Wh[:, D : 2 * D].T

    nbz_dev = np.ascontiguousarray((-bz_eff).reshape(HT, P).T).astype(np.float32)
    bh_dev = np.ascontiguousarray(bh_eff.reshape(HT, P).T).astype(np.float32)

    common = {"nbz": nbz_dev, "bh": bh_dev}
    if "bf16" in modes:

        def wdev(weff):  # [2D, H] -> [P, KT, HT, P]
            return weff.reshape(KT, P, HT, P).transpose(1, 0, 2, 3)

        common["w"] = np.ascontiguousarray(
            np.stack([wdev(Wz_eff), wdev(Wh_eff)], axis=1)
        ).astype(bf16)  # [P, 2, KT, HT, P]
    if "fp8" in modes:

        def wdev8(weff):  # [2D, H] -> [P, K2, HT, I2, P]; K = k2*256 + i*128 + p
            return weff.reshape(K2, I2, P, HT, P).transpose(2, 0, 3, 1, 4)

        common["w8"] = np.ascontiguousarray(
            np.stack([wdev8(Wz_eff), wdev8(Wh_eff)], axis=1)
        ).astype(f8np)  # [P, 2, K2, HT, I2, P]

    in_maps = []
    for c in range(NC):
        xt_c = x_tilde[c * BL : (c + 1) * BL]  # [BL, T, D]
        m_c = M[c * BL : (c + 1) * BL]
        feat = np.concatenate(
            [xt_c.transpose(2, 0, 1), m_c.transpose(2, 0, 1)], axis=0
        )  # [2D, BL, T]
        m = dict(common)
        if "bf16" in modes:
            m["inp"] = np.ascontiguousarray(
                feat.reshape(KT, P, NSB, SB, T).transpose(1, 2, 0, 4, 3)
            ).astype(bf16)  # [P, NSB, KT, T, SB]
        if "fp8" in modes:
            # [P, NSB, K2, I2, T, SB]; K = k2*256 + i*128 + p, col = t*SB + b
            m["inp8"] = np.ascontiguousarray(
                feat.reshape(K2, I2, P, NSB, SB, T).transpose(2, 3, 0, 1, 5, 4)
            ).astype(f8np)
        in_maps.append(m)
    return in_maps


def _finish(results, Wout, bout):
    h_all = np.empty((B, H), np.float32)
    for c in range(NC):
        ho = np.asarray(results[c]["hout"], np.float32)  # [P, HT, BL]
        h_all[c * BL : (c + 1) * BL] = ho.transpose(2, 1, 0).reshape(BL, H)

    wout = np.asarray(Wout, np.float32)
    logits = h_all @ wout[0] + np.asarray(bout, np.float32)[0]
    return (1.0 / (1.0 + np.exp(-logits))).astype(np.float32)


def kernel(X, M, input_means, gamma_x, Wz, bz, Wr, br, Wh, bh, Wout, bout):
    global _last_results
    mode = "fp8" if "fp8" in MAIN_PARTS else "bf16"
    in_maps = _prepare_in_maps(
        X, M, input_means, gamma_x, Wz, bz, Wh, bh, modes=(mode,)
    )
    nc = _build_bass(1, MAIN_PARTS)
    from concourse import bass_utils

    res = bass_utils.run_bass_kernel_spmd(
        nc,
        in_maps,
        core_ids=list(range(NC)),
        trace=False,
    )
    _last_results = res
    return _finish(res.results, Wout, bout)


# revision 47
# speedup vs baseline: 1.0741x; 1.0741x over previous
"""GRU-D forward on 8 Trainium2 NeuronCores (Bass/Tile kernel).

Key algebraic structure exploited:
  - The gates z_t, h~_t depend only on inputs (not on h), so all matmuls are
    parallel over T; only the elementwise blend h = (1-z)h + z*h~ is a
    recurrence, and it maps onto the DVE tensor_tensor_scan instruction
    (state = a*state - c' with a = sigmoid(-pre_z), c' = (a-1)*h~).
  - r_t is computed-but-unused in the reference -> Wr matmul skipped.
  - The xm block of inp = [x_tilde, xm, m] is constant -> folded into biases.

Sharding: data-parallel over batch (64 rows per core); weights replicated.
"""

import os
import sys

import numpy as np

sys.path.insert(0, "/opt/trn_rl_repo")

B, T, D, H = 512, 256, 256, 1024
NC = 8
BL = B // NC  # 64 batch rows per core
SB = 2  # batch elems per sub-batch (x T=256 -> 512 matmul rows)
NSB = BL // SB  # 32 sub-batches
KT = 4  # K tiles of 128 over 2D=512 contraction
K2 = 2  # fp8 DoubleRow K tiles of 256
I2 = 2  # DoubleRow plane pairing
HT = 8  # H tiles of 128
P = 128

# Engine/dtype configuration of the graded kernel.
MAIN_PARTS = ("mm", "act", "dve", "fp8")

_nc_cache = {}
_last_results = None


def _build_bass(repeat=1, parts=("mm", "act", "dve")):
    """Build the Bass program. repeat>1 wraps the whole computation in a
    hardware For loop executing it `repeat` times — used only for timing
    (overhead-cancelling slope measurement); the graded path uses repeat=1.
    parts: subset of {mm, act, dve} — engine-attribution experiments."""
    parts = frozenset(parts)
    key = (repeat, parts)
    if key in _nc_cache:
        return _nc_cache[key]
    from contextlib import ExitStack

    from concourse import bacc, mybir, tile

    nc = bacc.Bacc("TRN2", target_bir_lowering=False, debug=False, num_devices=NC)
    bf16 = mybir.dt.bfloat16
    f32 = mybir.dt.float32
    AF = mybir.ActivationFunctionType
    OP = mybir.AluOpType

    f8 = mybir.dt.float8e4
    if "fp8" in parts:
        if "ileav" in parts:
            in_d = nc.dram_tensor(
                "inp8i", [P, NSB, K2, T * SB, I2], f8, kind="ExternalInput"
            )
        else:
            in_d = nc.dram_tensor(
                "inp8", [P, NSB, K2, I2, T, SB], f8, kind="ExternalInput"
            )
        w_d = nc.dram_tensor("w8", [P, 2, K2, HT, I2, P], f8, kind="ExternalInput")
    else:
        in_d = nc.dram_tensor("inp", [P, NSB, KT, T, SB], bf16, kind="ExternalInput")
        w_d = nc.dram_tensor("w", [P, 2, KT, HT, P], bf16, kind="ExternalInput")
    nbz_d = nc.dram_tensor("nbz", [P, HT], f32, kind="ExternalInput")
    bh_d = nc.dram_tensor("bh", [P, HT], f32, kind="ExternalInput")
    hout_d = nc.dram_tensor("hout", [P, HT, BL], f32, kind="ExternalOutput")

    with tile.TileContext(nc) as tc:
        with (
            tc.tile_pool(name="const", bufs=1) as cpool,
            tc.tile_pool(name="inb", bufs=3) as ipool,
            tc.tile_pool(name="act", bufs=5) as apool,
            tc.tile_pool(name="zps", bufs=3, space="PSUM") as zpool,
            tc.tile_pool(name="hps", bufs=3, space="PSUM") as hpool,
        ):
            if "fp8" in parts:
                w_s = cpool.tile([P, 2, K2, HT, I2, P], f8)
            else:
                w_s = cpool.tile([P, 2, KT, HT, P], bf16)
            nbz_s = cpool.tile([P, HT], f32)
            bh_s = cpool.tile([P, HT], f32)
            if "dve" not in parts:
                hlast = cpool.tile([P, HT, BL], f32)
                nc.vector.memset(hlast[:], 0.0)
            cst = None
            if "xdep" in parts:  # timing probe: DVE reads constants, not ACT out
                cst = cpool.tile([P, HT, T, SB], bf16)
                nc.vector.memset(cst[:], 0.5)
            nc.sync.dma_start(w_s[:], w_d[:])
            nc.sync.dma_start(nbz_s[:], nbz_d[:])
            nc.sync.dma_start(bh_s[:], bh_d[:])

            with ExitStack() as rep_ctx:
                if repeat > 1:
                    rep_ctx.enter_context(tc.For_i(0, repeat, 1))
                _emit_body(nc, mybir, parts, ipool, apool, zpool, hpool,
                           in_d, w_s, nbz_s, bh_s, hout_d, cst)
            if "dve" not in parts:
                nc.sync.dma_start(hout_d[:], hlast[:])
    nc.compile()
    _nc_cache[key] = nc
    return nc


def _emit_body(nc, mybir, parts, ipool, apool, zpool, hpool, in_d, w_s, nbz_s, bh_s, hout_d, cst=None):
    bf16 = mybir.dt.bfloat16
    f8 = mybir.dt.float8e4
    f32 = mybir.dt.float32
    AF = mybir.ActivationFunctionType
    OP = mybir.AluOpType
    fp8 = "fp8" in parts
    ileav = "ileav" in parts
    for sb in range(NSB):
        if fp8 and ileav:
            in_s = ipool.tile([P, K2, T * SB, I2], f8, tag="in")
        elif fp8:
            in_s = ipool.tile([P, K2, I2, T * SB], f8, tag="in")
        else:
            in_s = ipool.tile([P, KT, T, SB], bf16, tag="in")
        nc.sync.dma_start(in_s[:], in_d[:, sb])
        # [P, HT, T, SB]: t-major so the d-prep diff runs in 2x DVE mode
        # (innermost SB stride-1 on both operands); scans tolerate stride-2.
        a_s = apool.tile([P, HT, T + 1, SB], bf16, tag="a")
        if "dve" in parts and cst is None:
            # lead slot multiplies a zero state but must be finite
            nc.vector.memset(a_s[:, :, 0, :], 0.0)
        ht_s = apool.tile([P, HT, T, SB], bf16, tag="ht")
        cp_s = apool.tile([P, HT, T, SB], bf16, tag="cp")
        so_s = apool.tile([P, HT, T, SB], bf16, tag="so")
        if "flat" in parts:  # timing-only: 1-D free APs for matmul stream
            in_f = ipool.tile([P, KT, T * SB], bf16, tag="inf")
            nc.sync.dma_start(in_f[:], in_d[:, sb])
            for j in range(HT):
                ps_zf = zpool.tile([P, T * SB], f32, tag="zf")
                ps_hf = hpool.tile([P, T * SB], f32, tag="hf")
                for k in range(KT):
                    nc.tensor.matmul(
                        ps_zf[:], w_s[:, 0, k, j, :], in_f[:, k],
                        start=(k == 0), stop=(k == KT - 1),
                    )
                for k in range(KT):
                    nc.tensor.matmul(
                        ps_hf[:], w_s[:, 1, k, j, :], in_f[:, k],
                        start=(k == 0), stop=(k == KT - 1),
                    )
            continue
        for j in range(HT):
            ps_z = zpool.tile([P, T, SB], f32, tag="z")
            ps_h = hpool.tile([P, T, SB], f32, tag="h")
            if fp8 and "mm" in parts:
                DR = mybir.MatmulPerfMode.DoubleRow
                for gate, ps in ((0, ps_z), (1, ps_h)):
                    for k2 in range(K2):
                        rhs = in_s[:, k2]
                        if ileav:  # [128, N, 2] -> [128, 2, N], pair stride 1
                            rhs = rhs.transpose([0, 2, 1])
                        nc.tensor.matmul(
                            ps[:],
                            w_s[:, gate, k2, j],
                            rhs,
                            start=(k2 == 0),
                            stop=(k2 == K2 - 1),
                            perf_mode=DR,
                        )
            elif "mm" in parts:
                fixed_w = "w0" in parts  # timing-only: no weight switching
                bigacc = "bigacc" in parts  # timing-only: one huge accum group
                for k in range(KT):
                    nc.tensor.matmul(
                        ps_z[:],
                        w_s[:, 0, 0, 0, :] if fixed_w else w_s[:, 0, k, j, :],
                        in_s[:, k],
                        start=(j == 0 and k == 0) if bigacc else (k == 0),
                        stop=(j == HT - 1 and k == KT - 1) if bigacc else (k == KT - 1),
                        skip_group_check=bigacc,
                    )
                for k in range(KT):
                    nc.tensor.matmul(
                        ps_h[:],
                        w_s[:, 0, 0, 0, :] if fixed_w else w_s[:, 1, k, j, :],
                        in_s[:, k],
                        start=(j == 0 and k == 0) if bigacc else (k == 0),
                        stop=(j == HT - 1 and k == KT - 1) if bigacc else (k == KT - 1),
                        skip_group_check=bigacc,
                    )
            if "act" in parts:
                # a = 1 - z = sigmoid(-(pre_z + bz)); PSUM read transposed
                # (t,b)->(b,t) so SBUF activations are t-contiguous per batch
                nc.scalar.activation(
                    a_s[:, j, 1 : T + 1],
                    ps_z[:],
                    AF.Sigmoid,
                    bias=nbz_s[:, j : j + 1],
                    scale=-1.0,
                )
                nc.scalar.activation(
                    ht_s[:, j],
                    ps_h[:],
                    AF.Tanh,
                    bias=bh_s[:, j : j + 1],
                    scale=1.0,
                )
            if "dve" in parts:
                if cst is not None:  # timing probe: cut the ACT->DVE dependency
                    a_v, ht_v = cst, cst
                else:
                    a_v, ht_v = a_s, ht_s
                # w-transform of h = a*h + (1-a)*h~ : with u_t := h~_t - h_{t-1},
                #   u_{t+1} = a_t*u_t - d_t,  d_t := h~_t - h~_{t+1},  u_1 = h~_1
                # and finally h_T = h~_T - a_T * u_T.  This avoids the (1-a)*h~
                # tensor product (STT is 1x rate; the shifted diff runs at 2x).
                nc.vector.tensor_scalar_mul(cp_s[:, j, 0], ht_v[:, j, 0], -1.0)
                nc.vector.tensor_sub(
                    cp_s[:, j, 1:T],
                    ht_v[:, j, 0 : T - 1],
                    ht_v[:, j, 1:T],
                )
                for b in range(SB):
                    nc.vector.tensor_tensor_scan(
                        so_s[:, j, :, b],
                        a_v[:, j, 0:T, b],
                        cp_s[:, j, :, b],
                        0.0,
                        op0=OP.mult,
                        op1=OP.subtract,
                    )
        if "dve" in parts:
            # h_T = h~_T - a_T * u_T on the strided last-column views, then DMA
            hf = apool.tile([P, HT, SB], f32, tag="hf")
            av2 = cst if cst is not None else a_s
            hv2 = cst if cst is not None else ht_s
            nc.vector.tensor_tensor(
                hf[:], av2[:, :, T - (0 if av2 is not cst else 1)] if False else av2[:, :, T if av2 is not cst else T - 1], so_s[:, :, T - 1], op=OP.mult
            )
            nc.vector.tensor_sub(hf[:], hv2[:, :, T - 1], hf[:])
            nc.sync.dma_start(hout_d[:, :, sb * SB : (sb + 1) * SB], hf[:])


def _prepare_in_maps(X, M, input_means, gamma_x, Wz, bz, Wh, bh, modes=("fp8",)):
    import ml_dtypes

    from concourse import mybir

    bf16 = ml_dtypes.bfloat16
    f8np = mybir.dt.np(mybir.dt.float8e4)
    X = np.asarray(X, np.float32)
    M = np.asarray(M, np.float32)
    xm = np.asarray(input_means, np.float32)
    gx = np.asarray(gamma_x, np.float32)
    Wz = np.asarray(Wz, np.float32)
    Wh = np.asarray(Wh, np.float32)
    bz = np.asarray(bz, np.float32)
    bhv = np.asarray(bh, np.float32)

    # x_tilde (exact, fp32, handles arbitrary gamma_x / non-binary M)
    g = np.exp(-gx * (1.0 - M))
    x_hat = M * X + (1.0 - M) * xm
    x_tilde = g * x_hat + (1.0 - g) * xm  # [B, T, D]

    # fold the constant xm block into the biases; drop unused Wr entirely
    Wz_eff = np.concatenate([Wz[:, :D], Wz[:, 2 * D :]], axis=1).T  # [2D, H]
    Wh_eff = np.concatenate([Wh[:, :D], Wh[:, 2 * D :]], axis=1).T
    bz_eff = bz + xm @ Wz[:, D : 2 * D].T
    bh_eff = bhv + xm @ Wh[:, D : 2 * D].T

    nbz_dev = np.ascontiguousarray((-bz_eff).reshape(HT, P).T).astype(np.float32)
    bh_dev = np.ascontiguousarray(bh_eff.reshape(HT, P).T).astype(np.float32)

    common = {"nbz": nbz_dev, "bh": bh_dev}
    if "bf16" in modes:

        def wdev(weff):  # [2D, H] -> [P, KT, HT, P]
            return weff.reshape(KT, P, HT, P).transpose(1, 0, 2, 3)

        common["w"] = np.ascontiguousarray(
            np.stack([wdev(Wz_eff), wdev(Wh_eff)], axis=1)
        ).astype(bf16)  # [P, 2, KT, HT, P]
    if "fp8" in modes or "fp8i" in modes:

        def wdev8(weff):  # [2D, H] -> [P, K2, HT, I2, P]; K = k2*256 + i*128 + p
            return weff.reshape(K2, I2, P, HT, P).transpose(2, 0, 3, 1, 4)

        common["w8"] = np.ascontiguousarray(
            np.stack([wdev8(Wz_eff), wdev8(Wh_eff)], axis=1)
        ).astype(f8np)  # [P, 2, K2, HT, I2, P]

    in_maps = []
    for c in range(NC):
        xt_c = x_tilde[c * BL : (c + 1) * BL]  # [BL, T, D]
        m_c = M[c * BL : (c + 1) * BL]
        feat = np.concatenate(
            [xt_c.transpose(2, 0, 1), m_c.transpose(2, 0, 1)], axis=0
        )  # [2D, BL, T]
        m = dict(common)
        if "bf16" in modes:
            m["inp"] = np.ascontiguousarray(
                feat.reshape(KT, P, NSB, SB, T).transpose(1, 2, 0, 4, 3)
            ).astype(bf16)  # [P, NSB, KT, T, SB]
        if "fp8" in modes:
            # [P, NSB, K2, I2, T, SB]; K = k2*256 + i*128 + p, col = t*SB + b
            m["inp8"] = np.ascontiguousarray(
                feat.reshape(K2, I2, P, NSB, SB, T).transpose(2, 3, 0, 1, 5, 4)
            ).astype(f8np)
        if "fp8i" in modes:
            # [P, NSB, K2, T*SB, I2]: pair-interleaved K planes
            m["inp8i"] = np.ascontiguousarray(
                feat.reshape(K2, I2, P, NSB, SB, T)
                .transpose(2, 3, 0, 5, 4, 1)
                .reshape(P, NSB, K2, T * SB, I2)
            ).astype(f8np)
        in_maps.append(m)
    return in_maps


def _finish(results, Wout, bout):
    h_all = np.empty((B, H), np.float32)
    for c in range(NC):
        ho = np.asarray(results[c]["hout"], np.float32)  # [P, HT, BL]
        h_all[c * BL : (c + 1) * BL] = ho.transpose(2, 1, 0).reshape(BL, H)

    wout = np.asarray(Wout, np.float32)
    logits = h_all @ wout[0] + np.asarray(bout, np.float32)[0]
    return (1.0 / (1.0 + np.exp(-logits))).astype(np.float32)


def kernel(X, M, input_means, gamma_x, Wz, bz, Wr, br, Wh, bh, Wout, bout):
    global _last_results
    if "fp8" in MAIN_PARTS:
        mode = "fp8i" if "ileav" in MAIN_PARTS else "fp8"
    else:
        mode = "bf16"
    in_maps = _prepare_in_maps(
        X, M, input_means, gamma_x, Wz, bz, Wh, bh, modes=(mode,)
    )
    nc = _build_bass(1, MAIN_PARTS)
    from concourse import bass_utils

    res = bass_utils.run_bass_kernel_spmd(
        nc,
        in_maps,
        core_ids=list(range(NC)),
        trace=False,
    )
    _last_results = res
    return _finish(res.results, Wout, bout)
